# revision 5
# baseline (speedup 1.0000x reference)
"""Trainium2 Bass kernel for nn_CSTri (membrane / cloth triangle energy).

Math: the reference computes, per face, the eigenvalues of the 2x2
Cauchy-Green tensor C = F^T F built from an intrinsic 2D basis of the
reference triangle.  C is similar to G @ R^{-1} where G and R are the 2x2
edge Gram matrices of the deformed / reference triangle:

    G = [[|e0|^2, e0.e1], [e0.e1, |e1|^2]]   (deformed edges, per batch)
    R = same for reference edges              (per face, batch-independent)

so eig(C) = eig(G R^{-1}):  tr = (s00 r11 - 2 s01 r01 + s11 r00)/detR,
det = detG/detR.  All the cross products / normalisations in the reference
cancel, and f_rest_areas = sqrt(detR)/2.

Tension-field relaxation is handled branch-free: with
eig_max := max(t + rh, 1), emt := eig_max^{-1/2}, emin := max(t - rh, emt),
the energy-density-plus-mu  en0 = 0.5*mu*(eig_max+emin) + (lam/8*L - mu/2)*L
(L = ln(eig_max*emin)) equals exactly mu when the clamp engages
(eig_max' = 1 -> L = 0, emin = 1), so  energy_density = en0 - mu  is exactly
0 for compressed faces -- no mask needed.  The constant -mu is folded into
the final host-side reduction via sum(w).

Sharding: faces = arange(V).reshape(F, 3), so face f uses exactly vertices
3f, 3f+1, 3f+2 -- the "gather" is a reshape and an even split of the face
dim across 8 cores is a contiguous slice of the vertex dim.

Per core layout: [128 partitions, 512 faces] fp32 tiles; the raw 9 floats
per face stay interleaved in SBUF and are read with stride-9 access
patterns (free for fp32 1x DVE ops).
"""

import numpy as np

B, V, F, M = 8, 1572864, 524288, 8
FC = F // M            # 65536 faces per core
VC = V // M            # 196608 vertices per core
P, W = 128, 512        # FC = P * W
POISSON = 0.33
EPS = 1e-15
LN_HALF = -0.6931471805599453

LAST_RESULTS = None    # BassKernelResults of the most recent run (for test.py)


def _split_multi_waits(nc, mybir):
    """Walrus in this image caps sync waits at 1/instruction (2 for
    EventSemaphore); Tile can emit more.  Move extras onto NoOps."""
    for fn in nc.m.functions:
        for bb in fn.blocks:
            insts = bb.instructions
            new_list = []
            changed = False
            for inst in insts:
                si = inst.sync_info
                waits = list(si.on_wait) if si is not None and si.on_wait else []
                cap = 2 if inst.opcode == "EventSemaphore" else 1
                if len(waits) > cap:
                    extra, keep = waits[:-cap], waits[-cap:]
                    for k, w in enumerate(extra):
                        new_list.append(mybir.InstNoOp(
                            name=f"{inst.name}_wsplit{k}",
                            sync_info=mybir.SyncInfo(on_wait=[w], on_update=[]),
                            engine=inst.engine,
                            bass_nofuse=True,
                        ))
                    si.on_wait = keep
                    inst.sync_info = si
                    changed = True
                new_list.append(inst)
            if changed:
                insts[:] = new_list


def _build(mu, lam, waitsplit=True):
    import concourse.bass as bass
    import concourse.mybir as mybir
    from concourse.tile import TileContext

    dt = mybir.dt.float32
    Alu = mybir.AluOpType
    Act = mybir.ActivationFunctionType

    nc = bass.Bass()
    verts = nc.declare_dram_parameter("verts", [B, VC, 3], dt, isOutput=False)
    vref = nc.declare_dram_parameter("vref", [VC, 3], dt, isOutput=False)
    thick = nc.declare_dram_parameter("thick", [FC], dt, isOutput=False)
    out = nc.declare_dram_parameter("out", [P, 16], dt, isOutput=True)

    with TileContext(nc) as tc:
        with (
            tc.tile_pool(name="xp", bufs=2) as xp,
            tc.tile_pool(name="coef", bufs=1) as coef,
            tc.tile_pool(name="sc", bufs=1) as sc,
        ):
            def T(tag):
                return sc.tile([P, W], dt, tag=tag, name=tag)

            def edges_and_gram(Xtile, pfx):
                """Xtile: [P, 9W] interleaved verts -> (s00, s01, s11)."""
                Xv = Xtile.rearrange("p (w c) -> p w c", c=9)
                e = []
                for vi, c in ((1, 0), (1, 1), (1, 2), (2, 0), (2, 1), (2, 2)):
                    t = T(f"{pfx}e{vi}{c}")
                    nc.vector.tensor_sub(t, Xv[:, :, 3 * vi + c], Xv[:, :, c])
                    e.append(t)
                e0x, e0y, e0z, e1x, e1y, e1z = e
                q = []
                for i, t in enumerate(e):
                    s = T(f"{pfx}q{i}")
                    nc.scalar.activation(s, t, Act.Square)
                    q.append(s)
                # s01 = e0 . e1
                m1, m2 = T(f"{pfx}m1"), T(f"{pfx}m2")
                nc.vector.tensor_mul(m1, e0x, e1x)
                nc.vector.tensor_mul(m2, e0y, e1y)
                nc.vector.tensor_add(m1, m1, m2)
                nc.vector.tensor_mul(m2, e0z, e1z)
                s01 = T(f"{pfx}s01")
                nc.vector.tensor_add(s01, m1, m2)
                # s00, s11
                s00, s11 = T(f"{pfx}s00"), T(f"{pfx}s11")
                nc.vector.tensor_add(s00, q[0], q[1])
                nc.vector.tensor_add(s00, s00, q[2])
                nc.vector.tensor_add(s11, q[3], q[4])
                nc.vector.tensor_add(s11, s11, q[5])
                return s00, s01, s11

            # ---------------- per-face reference coefficients ----------------
            Rt = coef.tile([P, 9 * W], dt, name="Rt")
            nc.sync.dma_start(out=Rt, in_=vref.rearrange("(p w) c -> p (w c)", p=P))
            TH = coef.tile([P, W], dt, name="TH")
            nc.sync.dma_start(out=TH, in_=thick.rearrange("(p w) -> p w", p=P))

            b_lnh = coef.tile([P, 1], dt, name="b_lnh")
            nc.vector.memset(b_lnh, LN_HALF)
            b_t1 = coef.tile([P, 1], dt, name="b_t1")
            nc.vector.memset(b_t1, -0.5 * mu)

            r00, r01, r11 = edges_and_gram(Rt, "r")
            z = T("rz")
            nc.vector.tensor_mul(z, r00, r11)
            zz = T("rzz")
            nc.scalar.activation(zz, r01, Act.Square)
            detR = T("detR")
            nc.vector.tensor_sub(detR, z, zz)
            rec = T("rrec")
            nc.vector.reciprocal(rec, detR)

            p0, p1, p2, qc, Wf = (coef.tile([P, W], dt, tag=f"c{i}", name=f"c{i}") for i in range(5))
            # p0 = r11/(2 detR), p2 = r00/(2 detR), p1 = -r01/detR, qc = 1/(4 detR)
            nc.vector.scalar_tensor_tensor(p0, r11, 0.5, rec, Alu.mult, Alu.mult)
            nc.vector.scalar_tensor_tensor(p2, r00, 0.5, rec, Alu.mult, Alu.mult)
            nc.vector.scalar_tensor_tensor(p1, r01, -1.0, rec, Alu.mult, Alu.mult)
            nc.vector.tensor_scalar_mul(qc, rec, 0.25)
            # Wf = 0.5*sqrt(detR)*thickness   (sqrt via exp(0.5 ln + ln 0.5))
            ld = T("rld")
            nc.scalar.activation(ld, detR, Act.Ln)
            ex = T("rex")
            nc.scalar.activation(ex, ld, Act.Exp, bias=b_lnh, scale=0.5)
            nc.vector.tensor_mul(Wf, ex, TH)

            out_t = coef.tile([P, 16], dt, name="out_t")
            nc.vector.memset(out_t, 0.0)
            nc.vector.tensor_reduce(out_t[:, 8:9], Wf, mybir.AxisListType.X, Alu.add)

            # ---------------- per-batch face energies ----------------
            for b in range(B):
                X = xp.tile([P, 9 * W], dt, tag="X", name="X")
                nc.sync.dma_start(
                    out=X, in_=verts[b].rearrange("(p w) c -> p (w c)", p=P)
                )
                s00, s01, s11 = edges_and_gram(X, "b")

                # t = tr/2 = p0 s00 + p1 s01 + p2 s11
                ta, tb_ = T("ta"), T("tb")
                nc.vector.tensor_mul(ta, s00, p0)
                nc.vector.tensor_mul(tb_, s01, p1)
                nc.vector.tensor_add(ta, ta, tb_)
                nc.vector.tensor_mul(tb_, s11, p2)
                t = T("t")
                nc.vector.tensor_add(t, ta, tb_)

                # d4 = det/4 = (s00 s11 - s01^2) * q
                z2 = T("z2")
                nc.vector.tensor_mul(z2, s00, s11)
                z1 = T("z1")
                nc.scalar.activation(z1, s01, Act.Square)
                nc.vector.tensor_sub(z2, z2, z1)
                d4 = T("d4")
                nc.vector.tensor_mul(d4, z2, qc)

                # rh = sqrt(max(t^2 - d4, EPS))
                u = T("u")
                nc.scalar.activation(u, t, Act.Square)
                ap_ = T("ap")
                nc.vector.tensor_sub(ap_, u, d4)
                nc.vector.tensor_scalar_max(ap_, ap_, EPS)
                la = T("la")
                nc.scalar.activation(la, ap_, Act.Ln)
                rh = T("rh")
                nc.scalar.activation(rh, la, Act.Exp, scale=0.5)

                emin = T("emin")
                nc.vector.tensor_sub(emin, t, rh)          # eig_min
                emax = T("emax")
                nc.vector.tensor_add(emax, t, rh)
                nc.vector.tensor_scalar_max(emax, emax, 1.0)  # relaxation clamp

                lm = T("lm")
                nc.scalar.activation(lm, emax, Act.Ln)
                emt = T("emt")
                nc.scalar.activation(emt, lm, Act.Exp, scale=-0.5)  # emax^-1/2
                nc.vector.tensor_max(emin, emin, emt)

                iic = T("iic")
                nc.vector.tensor_mul(iic, emax, emin)
                L = T("L")
                nc.scalar.activation(L, iic, Act.Ln)
                t1 = T("t1")
                nc.scalar.activation(t1, L, Act.Identity,
                                     bias=b_t1, scale=0.125 * lam)
                t2 = T("t2")
                nc.vector.tensor_mul(t2, t1, L)
                sum1 = T("sum1")
                nc.vector.tensor_add(sum1, emax, emin)
                en0 = T("en0")
                nc.vector.scalar_tensor_tensor(en0, sum1, 0.5 * mu, t2,
                                               Alu.mult, Alu.add)
                enw = T("enw")
                nc.vector.scalar_tensor_tensor(
                    enw, en0, 1.0, Wf, Alu.mult, Alu.mult,
                    accum_out=out_t[:, b:b + 1],
                )

            nc.sync.dma_start(out=out[:, :], in_=out_t)

    if waitsplit:
        _split_multi_waits(nc, mybir)
    return nc


def kernel(vertices, vertices_ref, faces, youngmoduli, thicknesses):
    import os
    from concourse.bass_utils import run_bass_kernel_spmd

    vertices = np.asarray(vertices)
    vertices_ref = np.asarray(vertices_ref)
    faces = np.asarray(faces)
    thicknesses = np.asarray(thicknesses)
    assert vertices.shape == (B, V, 3) and vertices_ref.shape == (V, 3)
    assert faces.shape == (F, 3)
    if not np.array_equal(faces, np.arange(V, dtype=faces.dtype).reshape(F, 3)):
        raise NotImplementedError("kernel assumes faces == arange(V).reshape(F,3)")

    ym = float(np.asarray(youngmoduli).reshape(-1)[0])
    mu = ym / (2.0 * (1.0 + POISSON))
    lam = ym * POISSON / ((1.0 + POISSON) * (1.0 - 2.0 * POISSON))

    nc = _build(mu, lam)

    in_maps = []
    for m in range(M):
        in_maps.append({
            "verts": np.ascontiguousarray(
                vertices[:, m * VC:(m + 1) * VC, :], dtype=np.float32),
            "vref": np.ascontiguousarray(
                vertices_ref[m * VC:(m + 1) * VC, :], dtype=np.float32),
            "thick": np.ascontiguousarray(
                thicknesses[m * FC:(m + 1) * FC], dtype=np.float32),
        })

    trace = os.environ.get("KERNEL_TRACE", "0") == "1"
    res = run_bass_kernel_spmd(nc, in_maps, core_ids=list(range(M)), trace=trace)
    global LAST_RESULTS
    LAST_RESULTS = res

    acc = np.zeros(B, dtype=np.float64)
    wsum = 0.0
    for m in range(M):
        o = res.results[m]["out"].astype(np.float64)
        acc += o[:, :B].sum(axis=0)
        wsum += o[:, 8].sum()
    energies = acc - mu * wsum
    return energies.astype(np.float32)


# revision 9
# speedup vs baseline: 1.0642x; 1.0642x over previous
"""Trainium2 Bass kernel for nn_CSTri (membrane / cloth triangle energy).

Math: the reference computes, per face, the eigenvalues of the 2x2
Cauchy-Green tensor C = F^T F built from an intrinsic 2D basis of the
reference triangle.  C is similar to G @ R^{-1} where G and R are the 2x2
edge Gram matrices of the deformed / reference triangle:

    G = [[|e0|^2, e0.e1], [e0.e1, |e1|^2]]   (deformed edges, per batch)
    R = same for reference edges              (per face, batch-independent)

so eig(C) = eig(G R^{-1}):  tr = (s00 r11 - 2 s01 r01 + s11 r00)/detR,
det = detG/detR.  All the cross products / normalisations in the reference
cancel, and f_rest_areas = sqrt(detR)/2.

Tension-field relaxation is handled branch-free: with
eig_max := max(t + rh, 1), emt := eig_max^{-1/2}, emin := max(t - rh, emt),
the energy-density-plus-mu  en0 = 0.5*mu*(eig_max+emin) + (lam/8*L - mu/2)*L
(L = ln(eig_max*emin)) equals exactly mu when the clamp engages
(eig_max' = 1 -> L = 0, emin = 1), so  energy_density = en0 - mu  is exactly
0 for compressed faces -- no mask needed.  The constant -mu is folded into
the final host-side reduction via sum(w).

Sharding: faces = arange(V).reshape(F, 3), so face f uses exactly vertices
3f, 3f+1, 3f+2 -- the "gather" is a reshape and an even split of the face
dim across 8 cores is a contiguous slice of the vertex dim.

Per core layout: [128 partitions, 512 faces] fp32 tiles; the raw 9 floats
per face stay interleaved in SBUF and are read with stride-9 access
patterns (free for fp32 1x DVE ops).
"""

import numpy as np

B, V, F, M = 8, 1572864, 524288, 8
FC = F // M            # 65536 faces per core
VC = V // M            # 196608 vertices per core
P, W = 128, 512        # FC = P * W
POISSON = 0.33
EPS = 1e-15
LN_HALF = -0.6931471805599453

LAST_RESULTS = None    # BassKernelResults of the most recent run (for test.py)


def _split_multi_waits(nc, mybir):
    """Walrus in this image caps sync waits at 1/instruction (2 for
    EventSemaphore); Tile can emit more.  Move extras onto NoOps."""
    for fn in nc.m.functions:
        for bb in fn.blocks:
            insts = bb.instructions
            new_list = []
            changed = False
            for inst in insts:
                si = inst.sync_info
                waits = list(si.on_wait) if si is not None and si.on_wait else []
                cap = 2 if inst.opcode == "EventSemaphore" else 1
                if len(waits) > cap:
                    extra, keep = waits[:-cap], waits[-cap:]
                    for k, w in enumerate(extra):
                        new_list.append(mybir.InstNoOp(
                            name=f"{inst.name}_wsplit{k}",
                            sync_info=mybir.SyncInfo(on_wait=[w], on_update=[]),
                            engine=inst.engine,
                            bass_nofuse=True,
                        ))
                    si.on_wait = keep
                    inst.sync_info = si
                    changed = True
                new_list.append(inst)
            if changed:
                insts[:] = new_list


def _build(mu, lam, waitsplit=True, bf16_tail=False):
    import concourse.bass as bass
    import concourse.mybir as mybir
    from concourse.tile import TileContext

    dt = mybir.dt.float32
    dtt = mybir.dt.bfloat16 if bf16_tail else dt
    Alu = mybir.AluOpType
    Act = mybir.ActivationFunctionType

    nc = bass.Bass()
    if bf16_tail:
        nc._allow_low_precision_reason = "bf16 energy tail; face sums accumulate in fp32 accum_out"
    verts = nc.declare_dram_parameter("verts", [B, VC, 3], dt, isOutput=False)
    vref = nc.declare_dram_parameter("vref", [VC, 3], dt, isOutput=False)
    thick = nc.declare_dram_parameter("thick", [FC], dt, isOutput=False)
    out = nc.declare_dram_parameter("out", [P, 16], dt, isOutput=True)

    with TileContext(nc) as tc:
        with (
            tc.tile_pool(name="xp", bufs=2) as xp,
            tc.tile_pool(name="coef", bufs=1) as coef,
            tc.tile_pool(name="sc", bufs=1) as sc,
        ):
            def T(tag, d=dt):
                return sc.tile([P, W], d, tag=tag, name=tag)

            def edges_and_gram(Xtile, pfx, sdt=dt):
                """Xtile: [P, 9W] interleaved verts -> (s00, s01, s11).

                Blocked layout: one strided sub produces e_int [P,(a=2,w,c=3)]
                (reads are 3-contiguous runs), one ACT square, one dense mul
                for e0*e1, then tensor_reduce over the innermost c=3.
                """
                Xq = Xtile.rearrange("p (w v c) -> p v w c", v=3, c=3)
                e_int = sc.tile([P, 6 * W], dt, tag=f"{pfx}ei", name=f"{pfx}ei")
                ev = e_int.rearrange("p (a w c) -> p a w c", a=2, c=3)
                v0 = Xq[:, 0, :, :]
                v0b = bass.AP(tensor=v0.tensor, offset=v0.offset,
                              ap=[v0.ap[0], [0, 2]] + list(v0.ap[1:]))
                nc.vector.tensor_sub(ev, Xq[:, 1:3, :, :], v0b)

                q_int = sc.tile([P, 6 * W], dt, tag=f"{pfx}qi", name=f"{pfx}qi")
                nc.scalar.activation(q_int, e_int, Act.Square)
                qv = q_int.rearrange("p (a w c) -> p a w c", a=2, c=3)

                m_int = sc.tile([P, 3 * W], dt, tag=f"{pfx}mi", name=f"{pfx}mi")
                mv = m_int.rearrange("p (w c) -> p w c", c=3)
                nc.vector.tensor_mul(mv, ev[:, 0], ev[:, 1])

                s_pair = sc.tile([P, 2 * W], sdt, tag=f"{pfx}sp", name=f"{pfx}sp")
                spv = s_pair.rearrange("p (a w) -> p a w", a=2)
                nc.vector.tensor_reduce(spv, qv, mybir.AxisListType.X, Alu.add)
                s01 = T(f"{pfx}s01", sdt)
                nc.vector.tensor_reduce(s01, mv, mybir.AxisListType.X, Alu.add)
                return spv[:, 0], s01, spv[:, 1]

            # ---------------- per-face reference coefficients ----------------
            Rt = coef.tile([P, 9 * W], dt, name="Rt")
            nc.sync.dma_start(out=Rt, in_=vref.rearrange("(p w) c -> p (w c)", p=P))
            TH = coef.tile([P, W], dt, name="TH")
            nc.sync.dma_start(out=TH, in_=thick.rearrange("(p w) -> p w", p=P))

            b_lnh = coef.tile([P, 1], dt, name="b_lnh")
            nc.vector.memset(b_lnh, LN_HALF)
            b_t1 = coef.tile([P, 1], dt, name="b_t1")
            nc.vector.memset(b_t1, -0.5 * mu)

            r00, r01, r11 = edges_and_gram(Rt, "r")
            z = T("rz")
            nc.vector.tensor_mul(z, r00, r11)
            zz = T("rzz")
            nc.scalar.activation(zz, r01, Act.Square)
            detR = T("detR")
            nc.vector.tensor_sub(detR, z, zz)
            rec = T("rrec")
            nc.vector.reciprocal(rec, detR)

            p0, p1, p2, qc = (coef.tile([P, W], dtt, tag=f"c{i}", name=f"c{i}") for i in range(4))
            Wf = coef.tile([P, W], dt, tag="c4", name="c4")
            # p0 = r11/(2 detR), p2 = r00/(2 detR), p1 = -r01/detR, qc = 1/(4 detR)
            nc.vector.scalar_tensor_tensor(p0, r11, 0.5, rec, Alu.mult, Alu.mult)
            nc.vector.scalar_tensor_tensor(p2, r00, 0.5, rec, Alu.mult, Alu.mult)
            nc.vector.scalar_tensor_tensor(p1, r01, -1.0, rec, Alu.mult, Alu.mult)
            nc.vector.tensor_scalar_mul(qc, rec, 0.25)
            # Wf = 0.5*sqrt(detR)*thickness   (sqrt via exp(0.5 ln + ln 0.5))
            ld = T("rld")
            nc.scalar.activation(ld, detR, Act.Ln)
            ex = T("rex")
            nc.scalar.activation(ex, ld, Act.Exp, bias=b_lnh, scale=0.5)
            nc.vector.tensor_mul(Wf, ex, TH)

            out_t = coef.tile([P, 16], dt, name="out_t")
            nc.vector.memset(out_t, 0.0)
            nc.vector.tensor_reduce(out_t[:, 8:9], Wf, mybir.AxisListType.X, Alu.add)

            # ---------------- per-batch face energies ----------------
            for b in range(B):
                X = xp.tile([P, 9 * W], dt, tag="X", name="X")
                nc.sync.dma_start(
                    out=X, in_=verts[b].rearrange("(p w) c -> p (w c)", p=P)
                )
                s00, s01, s11 = edges_and_gram(X, "b", sdt=dtt)

                # t = tr/2 = p0 s00 + p1 s01 + p2 s11
                ta, tb_ = T("ta", dtt), T("tb", dtt)
                nc.vector.tensor_mul(ta, s00, p0)
                nc.vector.tensor_mul(tb_, s01, p1)
                nc.vector.tensor_add(ta, ta, tb_)
                nc.vector.tensor_mul(tb_, s11, p2)
                t = T("t", dtt)
                nc.vector.tensor_add(t, ta, tb_)

                # d4 = det/4 = (s00 s11 - s01^2) * q
                z2 = T("z2", dtt)
                nc.vector.tensor_mul(z2, s00, s11)
                z1 = T("z1", dtt)
                nc.scalar.activation(z1, s01, Act.Square)
                nc.vector.tensor_sub(z2, z2, z1)
                d4 = T("d4", dtt)
                nc.vector.tensor_mul(d4, z2, qc)

                # rh = sqrt(max(t^2 - d4, EPS))
                u = T("u", dtt)
                nc.scalar.activation(u, t, Act.Square)
                ap_ = T("ap", dtt)
                nc.vector.tensor_sub(ap_, u, d4)
                nc.vector.tensor_scalar_max(ap_, ap_, EPS)
                la = T("la", dtt)
                nc.scalar.activation(la, ap_, Act.Ln)
                rh = T("rh", dtt)
                nc.scalar.activation(rh, la, Act.Exp, scale=0.5)

                emin = T("emin", dtt)
                nc.vector.tensor_sub(emin, t, rh)          # eig_min
                emax = T("emax", dtt)
                nc.vector.tensor_add(emax, t, rh)
                nc.vector.tensor_scalar_max(emax, emax, 1.0)  # relaxation clamp

                lm = T("lm", dtt)
                nc.scalar.activation(lm, emax, Act.Ln)
                emt = T("emt", dtt)
                nc.scalar.activation(emt, lm, Act.Exp, scale=-0.5)  # emax^-1/2
                nc.vector.tensor_max(emin, emin, emt)

                iic = T("iic", dtt)
                nc.vector.tensor_mul(iic, emax, emin)
                L = T("L", dtt)
                nc.scalar.activation(L, iic, Act.Ln)
                t1 = T("t1", dtt)
                nc.scalar.activation(t1, L, Act.Identity,
                                     bias=b_t1, scale=0.125 * lam)
                t2 = T("t2", dtt)
                nc.vector.tensor_mul(t2, t1, L)
                sum1 = T("sum1", dtt)
                nc.vector.tensor_add(sum1, emax, emin)
                en0 = T("en0", dtt)
                nc.vector.scalar_tensor_tensor(en0, sum1, 0.5 * mu, t2,
                                               Alu.mult, Alu.add)
                enw = T("enw", dtt)
                nc.vector.scalar_tensor_tensor(
                    enw, en0, 1.0, Wf, Alu.mult, Alu.mult,
                    accum_out=out_t[:, b:b + 1],
                )

            nc.sync.dma_start(out=out[:, :], in_=out_t)

    if waitsplit:
        _split_multi_waits(nc, mybir)
    return nc


def kernel(vertices, vertices_ref, faces, youngmoduli, thicknesses):
    import os
    from concourse.bass_utils import run_bass_kernel_spmd

    vertices = np.asarray(vertices)
    vertices_ref = np.asarray(vertices_ref)
    faces = np.asarray(faces)
    thicknesses = np.asarray(thicknesses)
    assert vertices.shape == (B, V, 3) and vertices_ref.shape == (V, 3)
    assert faces.shape == (F, 3)
    if not np.array_equal(faces, np.arange(V, dtype=faces.dtype).reshape(F, 3)):
        raise NotImplementedError("kernel assumes faces == arange(V).reshape(F,3)")

    ym = float(np.asarray(youngmoduli).reshape(-1)[0])
    mu = ym / (2.0 * (1.0 + POISSON))
    lam = ym * POISSON / ((1.0 + POISSON) * (1.0 - 2.0 * POISSON))

    import os as _os
    bf16_tail = _os.environ.get("KERNEL_BF16", "0") == "1"
    nc = _build(mu, lam, bf16_tail=bf16_tail)

    in_maps = []
    for m in range(M):
        in_maps.append({
            "verts": np.ascontiguousarray(
                vertices[:, m * VC:(m + 1) * VC, :], dtype=np.float32),
            "vref": np.ascontiguousarray(
                vertices_ref[m * VC:(m + 1) * VC, :], dtype=np.float32),
            "thick": np.ascontiguousarray(
                thicknesses[m * FC:(m + 1) * FC], dtype=np.float32),
        })

    trace = os.environ.get("KERNEL_TRACE", "0") == "1"
    res = run_bass_kernel_spmd(nc, in_maps, core_ids=list(range(M)), trace=trace)
    global LAST_RESULTS
    LAST_RESULTS = res

    acc = np.zeros(B, dtype=np.float64)
    wsum = 0.0
    for m in range(M):
        o = res.results[m]["out"].astype(np.float64)
        acc += o[:, :B].sum(axis=0)
        wsum += o[:, 8].sum()
    energies = acc - mu * wsum
    return energies.astype(np.float32)
